# revision 1
# baseline (speedup 1.0000x reference)
import numpy as np
import sys

sys.path.insert(0, "/opt/trn_rl_repo")

import ml_dtypes

BF16 = ml_dtypes.bfloat16

B, S, DIM = 4, 2048, 2048
N_HEADS, N_KV_HEADS, HEAD_DIM = 16, 4, 128
G = N_HEADS // N_KV_HEADS  # 4 q heads per kv head
EPS = 1.1920928955078125e-07
SCALE = 1.0 / np.sqrt(HEAD_DIM)
TOK = 2 * S  # tokens per core (a batch pair)
NT = TOK // 128  # 32 token tiles per core
NTB = S // 128  # 16 token tiles per batch

_CACHE = {}


def _build_graph():
    import concourse.bass as bass
    import concourse.mybir as mybir
    from concourse import bacc
    from concourse.tile import TileContext
    from concourse.masks import make_identity

    f32 = mybir.dt.float32
    bf16 = mybir.dt.bfloat16

    nc = bacc.Bacc()
    xT_d = nc.declare_dram_parameter("xT", [NT, 128, 16 * 128], bf16, isOutput=False)
    wqkv_d = nc.declare_dram_parameter("wqkvT", [DIM, 768], bf16, isOutput=False)
    wo_d = nc.declare_dram_parameter("woT", [512, DIM], bf16, isOutput=False)
    cosq_d = nc.declare_dram_parameter("cosq", [S, 128], f32, isOutput=False)
    sinq_d = nc.declare_dram_parameter("sinq", [S, 128], f32, isOutput=False)
    cosk_d = nc.declare_dram_parameter("cosk", [S, 128], f32, isOutput=False)
    sink_d = nc.declare_dram_parameter("sink", [S, 128], f32, isOutput=False)
    vis_d = nc.declare_dram_parameter("vis", [2, S], f32, isOutput=False)
    out_d = nc.declare_dram_parameter("out", [TOK, DIM], f32, isOutput=True)

    with TileContext(nc) as tc:
        with (
            tc.tile_pool(name="singles", bufs=1) as singles,
            tc.tile_pool(name="xin", bufs=2) as xin,
            tc.tile_pool(name="scr", bufs=2) as scr,
            tc.tile_pool(name="ptile", bufs=2) as ptile,
            tc.tile_pool(name="qtile", bufs=2) as qtile,
            tc.tile_pool(name="psum_big", bufs=2, space="PSUM") as psum_big,
            tc.tile_pool(name="psum_t", bufs=2, space="PSUM") as psum_t,
            tc.tile_pool(name="psum_o", bufs=2, space="PSUM") as psum_o,
        ):
            # ---- resident tiles ----
            ident = singles.tile([128, 128], bf16)
            make_identity(nc, ident)
            eps_sb = singles.tile([128, 1], f32)
            nc.vector.memset(eps_sb, EPS)
            wqkv_sb = singles.tile([128, 16, 768], bf16)
            nc.sync.dma_start(
                out=wqkv_sb, in_=wqkv_d[:, :].rearrange("(c p) f -> p c f", p=128)
            )
            wo_sb = singles.tile([128, 4, DIM], bf16)
            nc.sync.dma_start(
                out=wo_sb, in_=wo_d[:, :].rearrange("(c p) f -> p c f", p=128)
            )
            cosq_sb = singles.tile([128, NTB, 128], f32)
            sinq_sb = singles.tile([128, NTB, 128], f32)
            cosk_sb = singles.tile([128, NTB, 128], f32)
            sink_sb = singles.tile([128, NTB, 128], f32)
            for sb, d in (
                (cosq_sb, cosq_d),
                (sinq_sb, sinq_d),
                (cosk_sb, cosk_d),
                (sink_sb, sink_d),
            ):
                nc.sync.dma_start(
                    out=sb, in_=d[:, :].rearrange("(t p) d -> p t d", p=128)
                )
            vis_sb = singles.tile([128, NT], f32)
            nc.sync.dma_start(
                out=vis_sb, in_=vis_d[:, :].rearrange("b (t p) -> p (b t)", p=128)
            )
            q_tm = singles.tile([128, NT, G * 128], bf16)  # rope'd q, token-major
            k_tm = singles.tile([128, NT, 128], bf16)
            vv_sb = singles.tile([128, NT, 129], bf16)  # [v*vis | vis]
            kT_sb = singles.tile([128, 2, S], bf16)  # k transposed per batch

            # ---- stage 1: qkv matmul + rmsnorm + rope (token-major) ----
            for tt in range(NT):
                xt = xin.tile([128, 16, 128], bf16)
                nc.sync.dma_start(out=xt, in_=xT_d[tt, :, :])
                ps = psum_big.tile([128, 768], f32, tag="big")
                for kc in range(16):
                    nc.tensor.matmul(
                        ps[:, 0:512],
                        lhsT=xt[:, kc, :],
                        rhs=wqkv_sb[:, kc, 0:512],
                        start=(kc == 0),
                        stop=(kc == 15),
                    )
                    nc.tensor.matmul(
                        ps[:, 512:768],
                        lhsT=xt[:, kc, :],
                        rhs=wqkv_sb[:, kc, 512:768],
                        start=(kc == 0),
                        stop=(kc == 15),
                    )
                nrm = scr.tile([128, 768], f32, tag="nrm")
                nc.vector.tensor_copy(nrm, ps)
                ss = scr.tile([128, 5], f32, tag="ss")
                stdv = scr.tile([128, 5], f32, tag="stdv")
                rstd = scr.tile([128, 5], f32, tag="rstd")
                sqj = scr.tile([128, 128], f32, tag="sqj")
                t = tt % NTB
                for hh in range(5):
                    sl = nrm[:, hh * 128 : (hh + 1) * 128]
                    nc.scalar.activation(
                        out=sqj,
                        in_=sl,
                        func=mybir.ActivationFunctionType.Square,
                        accum_out=ss[:, hh : hh + 1],
                    )
                    nc.scalar.activation(
                        out=stdv[:, hh : hh + 1],
                        in_=ss[:, hh : hh + 1],
                        func=mybir.ActivationFunctionType.Sqrt,
                        bias=eps_sb[:, :],
                        scale=1.0 / 128.0,
                    )
                    nc.vector.reciprocal(rstd[:, hh : hh + 1], stdv[:, hh : hh + 1])
                    tmp1 = scr.tile([128, 128], f32, tag="tmp1")
                    nc.vector.tensor_scalar_mul(tmp1, sl, rstd[:, hh : hh + 1])
                    t1p = tmp1.rearrange("p (x two) -> p x two", two=2)
                    tmp2 = scr.tile([128, 128], f32, tag="tmp2")
                    t2p = tmp2.rearrange("p (x two) -> p x two", two=2)
                    nc.vector.tensor_scalar_mul(t2p[:, :, 0], t1p[:, :, 1], -1.0)
                    nc.vector.tensor_copy(t2p[:, :, 1], t1p[:, :, 0])
                    cs, sn = (cosq_sb, sinq_sb) if hh < 4 else (cosk_sb, sink_sb)
                    tmp3 = scr.tile([128, 128], f32, tag="tmp3")
                    nc.vector.tensor_mul(tmp3, tmp1, cs[:, t, :])
                    nc.vector.tensor_mul(tmp2, tmp2, sn[:, t, :])
                    dst = (
                        q_tm[:, tt, hh * 128 : (hh + 1) * 128]
                        if hh < 4
                        else k_tm[:, tt, :]
                    )
                    nc.vector.tensor_add(dst, tmp3, tmp2)
                # v with visibility folded in, plus the ones-column
                nc.vector.tensor_scalar_mul(
                    vv_sb[:, tt, 0:128], nrm[:, 640:768], vis_sb[:, tt : tt + 1]
                )

            nc.vector.tensor_copy(
                vv_sb[:, :, 128:129],
                vis_sb[:, :].rearrange("p (t one) -> p t one", one=1),
            )

            # ---- stage 2: transpose K per batch ----
            for b in range(2):
                for kt in range(NTB):
                    pt = psum_t.tile([128, 128], bf16, tag="pt")
                    nc.tensor.matmul(pt, lhsT=k_tm[:, b * NTB + kt, :], rhs=ident, is_transpose=True)
                    nc.vector.tensor_copy(
                        kT_sb[:, b, kt * 128 : (kt + 1) * 128], pt
                    )

            # ---- stage 3+4: attention + output projection per (b, qt) ----
            for b in range(2):
                for qt in range(NTB):
                    tt = b * NTB + qt
                    qT = qtile.tile([128, G, 128], bf16, tag="qT")
                    for hh in range(G):
                        pt = psum_t.tile([128, 128], bf16, tag="pt")
                        nc.tensor.matmul(
                            pt, lhsT=q_tm[:, tt, hh * 128 : (hh + 1) * 128], rhs=ident, is_transpose=True
                        )
                        nc.vector.tensor_copy(qT[:, hh, :], pt)
                    o_tmp = qtile.tile([128, G, 128], bf16, tag="o_tmp")
                    for hh in range(G):
                        p_sb = ptile.tile([128, S], bf16, tag="p_sb")
                        for half in range(2):
                            psc = psum_big.tile([128, 1024], f32, tag="big")
                            for kc in range(2):
                                col = half * 1024 + kc * 512
                                nc.tensor.matmul(
                                    psc[:, kc * 512 : (kc + 1) * 512],
                                    lhsT=qT[:, hh, :],
                                    rhs=kT_sb[:, b, col : col + 512],
                                    start=True,
                                    stop=True,
                                )
                            nc.scalar.activation(
                                out=p_sb[:, half * 1024 : (half + 1) * 1024],
                                in_=psc,
                                func=mybir.ActivationFunctionType.Exp,
                            )
                        pT = ptile.tile([128, NTB, 128], bf16, tag="pT")
                        for kt in range(NTB):
                            pt = psum_t.tile([128, 128], bf16, tag="pt")
                            nc.tensor.matmul(
                                pt, lhsT=p_sb[:, kt * 128 : (kt + 1) * 128], rhs=ident, is_transpose=True
                            )
                            nc.vector.tensor_copy(pT[:, kt, :], pt)
                        po = psum_o.tile([128, 129], f32, tag="po")
                        for kt in range(NTB):
                            nc.tensor.matmul(
                                po,
                                lhsT=pT[:, kt, :],
                                rhs=vv_sb[:, b * NTB + kt, :],
                                start=(kt == 0),
                                stop=(kt == NTB - 1),
                            )
                        rq = scr.tile([128, 1], f32, tag="rq")
                        nc.vector.reciprocal(rq, po[:, 128:129])
                        nc.vector.tensor_scalar_mul(rq, rq, vis_sb[:, tt : tt + 1])
                        nc.vector.tensor_scalar_mul(
                            o_tmp[:, hh, :], po[:, 0:128], rq
                        )
                    # output projection partial: y = o @ woT_slice
                    oT = qtile.tile([128, G, 128], bf16, tag="oT")
                    for ft in range(G):
                        pt = psum_t.tile([128, 128], bf16, tag="pt")
                        nc.tensor.matmul(pt, lhsT=o_tmp[:, ft, :], rhs=ident, is_transpose=True)
                        nc.vector.tensor_copy(oT[:, ft, :], pt)
                    for ncho in range(4):
                        py = psum_o.tile([128, 512], f32, tag="po")
                        for ft in range(4):
                            nc.tensor.matmul(
                                py,
                                lhsT=oT[:, ft, :],
                                rhs=wo_sb[:, ft, ncho * 512 : (ncho + 1) * 512],
                                start=(ft == 0),
                                stop=(ft == 3),
                            )
                        y_sb = scr.tile([128, 512], f32, tag="y")
                        nc.scalar.copy(y_sb, py)
                        nc.sync.dma_start(
                            out=out_d[
                                tt * 128 : (tt + 1) * 128,
                                ncho * 512 : (ncho + 1) * 512,
                            ],
                            in_=y_sb,
                        )
    nc.finalize()
    return nc


def _prep_inputs(x, wqkv, wo, q_norm_w, k_norm_w, freqs_cos, freqs_sin, vis_mask):
    """Build per-core input maps. Core c: kv head h=c%4, batch pair p=c//4."""
    cos = np.asarray(freqs_cos, np.float32)[:, 0, :]  # [S,128]
    sin = np.asarray(freqs_sin, np.float32)[:, 0, :]
    qw = np.asarray(q_norm_w, np.float32)
    kw = np.asarray(k_norm_w, np.float32)

    def swap_pairs(w):
        v = w.reshape(-1, 2)
        return np.stack([v[:, 1], v[:, 0]], axis=1).reshape(-1)

    cosq = (cos * qw[None, :] * SCALE).astype(np.float32)
    sinq = (sin * swap_pairs(qw)[None, :] * SCALE).astype(np.float32)
    cosk = (cos * kw[None, :]).astype(np.float32)
    sink = (sin * swap_pairs(kw)[None, :]).astype(np.float32)

    x = np.asarray(x, np.float32)
    wqkv = np.asarray(wqkv, np.float32)
    wo = np.asarray(wo, np.float32)
    visf = np.asarray(vis_mask).astype(np.float32)

    in_maps = []
    for c in range(8):
        h = c % 4
        p = c // 4
        xpair = np.concatenate([x[2 * p], x[2 * p + 1]], axis=0)  # [4096, 2048]
        xT = np.ascontiguousarray(
            xpair.reshape(NT, 128, 16, 128).transpose(0, 3, 2, 1).reshape(NT, 128, 2048)
        ).astype(BF16)
        wq = wqkv[512 * h : 512 * (h + 1)]  # [512, 2048]
        wk = wqkv[2048 + 128 * h : 2048 + 128 * (h + 1)]
        wv = wqkv[2560 + 128 * h : 2560 + 128 * (h + 1)]
        wslice = np.concatenate([wq, wk, wv], axis=0)  # [768, 2048]
        wqkvT = np.ascontiguousarray(wslice.T).astype(BF16)
        woT = np.ascontiguousarray(wo[:, 512 * h : 512 * (h + 1)].T).astype(BF16)
        vis = np.ascontiguousarray(visf[2 * p : 2 * p + 2])  # [2, S]
        in_maps.append(
            {
                "xT": xT,
                "wqkvT": wqkvT,
                "woT": woT,
                "cosq": cosq,
                "sinq": sinq,
                "cosk": cosk,
                "sink": sink,
                "vis": vis,
            }
        )
    return in_maps


def run_hw(in_maps, trace=False):
    from concourse.bass_utils import run_bass_kernel_spmd

    if "nc" not in _CACHE:
        _CACHE["nc"] = _build_graph()
    return run_bass_kernel_spmd(
        _CACHE["nc"], in_maps, core_ids=list(range(8)), trace=trace
    )


def kernel(x, wqkv, wo, q_norm_w, k_norm_w, freqs_cos, freqs_sin, vis_mask):
    in_maps = _prep_inputs(
        x, wqkv, wo, q_norm_w, k_norm_w, freqs_cos, freqs_sin, vis_mask
    )
    res = run_hw(in_maps)
    outs = [np.asarray(res.results[c]["out"], np.float32) for c in range(8)]
    full = np.zeros((B, S, DIM), np.float32)
    for p in range(2):
        acc = outs[4 * p + 0] + outs[4 * p + 1] + outs[4 * p + 2] + outs[4 * p + 3]
        full[2 * p] = acc[:S]
        full[2 * p + 1] = acc[S:]
    return full



# revision 2
# speedup vs baseline: 3.4202x; 3.4202x over previous
import numpy as np
import sys

sys.path.insert(0, "/opt/trn_rl_repo")

import ml_dtypes

BF16 = ml_dtypes.bfloat16

B, S, DIM = 4, 2048, 2048
N_HEADS, N_KV_HEADS, HEAD_DIM = 16, 4, 128
G = N_HEADS // N_KV_HEADS  # 4 q heads per kv head
EPS = 1.1920928955078125e-07
SCALE = 1.0 / np.sqrt(HEAD_DIM)
TOK = 2 * S  # tokens per core group (a batch pair)
NT = TOK // 128  # 32 token tiles per group
NTB = S // 128  # 16 token tiles per batch
NTS = NT // 4  # 8 token tiles per core slice (1024 tokens)
GROUPS = [[0, 1, 2, 3], [4, 5, 6, 7]]

_CACHE = {}


def _build_graph():
    import concourse.bass as bass
    import concourse.mybir as mybir
    from concourse import bacc
    from concourse.tile import TileContext
    from concourse.masks import make_identity

    f32 = mybir.dt.float32
    bf16 = mybir.dt.bfloat16

    nc = bacc.Bacc()
    # per-core token-quarter of the batch pair, pre-transposed to
    # feature-major tiles: [tile, feature-part, chunk*token]
    xTs_d = nc.declare_dram_parameter("xTs", [NTS, 128, 16 * 128], bf16, isOutput=False)
    wqkv_d = nc.declare_dram_parameter("wqkvT", [DIM, 768], bf16, isOutput=False)
    wo_d = nc.declare_dram_parameter("woT", [512, DIM], bf16, isOutput=False)
    cosq_d = nc.declare_dram_parameter("cosq", [S, 128], bf16, isOutput=False)
    sinq_d = nc.declare_dram_parameter("sinq", [S, 128], bf16, isOutput=False)
    cosk_d = nc.declare_dram_parameter("cosk", [S, 128], bf16, isOutput=False)
    sink_d = nc.declare_dram_parameter("sink", [S, 128], bf16, isOutput=False)
    vis_d = nc.declare_dram_parameter("vis", [2, S], f32, isOutput=False)
    out_d = nc.declare_dram_parameter("out", [TOK // 4, DIM], bf16, isOutput=True)

    # DRAM scratch (collectives can't touch I/O tensors)
    agx_in = nc.dram_tensor("agx_in", [NTS, 128, 16 * 128], bf16)
    agx_out = nc.dram_tensor("agx_out", [4, NTS, 128, 16 * 128], bf16)
    part_d = nc.dram_tensor("part", [TOK, DIM], bf16)
    rs_out = nc.dram_tensor("rs_out", [TOK // 4, DIM], bf16)

    with TileContext(nc) as tc:
        with (
            tc.tile_pool(name="singles", bufs=1) as singles,
            tc.tile_pool(name="xin", bufs=2) as xin,
            tc.tile_pool(name="scr", bufs=2) as scr,
            tc.tile_pool(name="ptile", bufs=2) as ptile,
            tc.tile_pool(name="qtile", bufs=2) as qtile,
            tc.tile_pool(name="psum_big", bufs=2, space="PSUM") as psum_big,
            tc.tile_pool(name="psum_t", bufs=2, space="PSUM") as psum_t,
            tc.tile_pool(name="psum_o", bufs=2, space="PSUM") as psum_o,
        ):
            # ---- stage 0: AllGather x across the 4-core group ----
            nc.sync.dma_start(out=agx_in[:, :, :], in_=xTs_d[:, :, :])
            nc.gpsimd.collective_compute(
                "AllGather",
                mybir.AluOpType.bypass,
                replica_groups=GROUPS,
                ins=[agx_in.ap().opt()],
                outs=[agx_out.ap().opt()],
            )

            # ---- resident tiles ----
            ident = singles.tile([128, 128], bf16)
            make_identity(nc, ident)
            eps_sb = singles.tile([128, 1], f32)
            nc.vector.memset(eps_sb, EPS)
            wqkv_sb = singles.tile([128, 16, 768], bf16)
            nc.sync.dma_start(
                out=wqkv_sb, in_=wqkv_d[:, :].rearrange("(c p) f -> p c f", p=128)
            )
            wo_sb = singles.tile([128, 4, DIM], bf16)
            nc.sync.dma_start(
                out=wo_sb, in_=wo_d[:, :].rearrange("(c p) f -> p c f", p=128)
            )
            cosq_sb = singles.tile([128, NTB, 128], bf16)
            sinq_sb = singles.tile([128, NTB, 128], bf16)
            cosk_sb = singles.tile([128, NTB, 128], bf16)
            sink_sb = singles.tile([128, NTB, 128], bf16)
            for sb, d in (
                (cosq_sb, cosq_d),
                (sinq_sb, sinq_d),
                (cosk_sb, cosk_d),
                (sink_sb, sink_d),
            ):
                nc.sync.dma_start(
                    out=sb, in_=d[:, :].rearrange("(t p) d -> p t d", p=128)
                )
            vis_sb = singles.tile([128, NT], f32)
            nc.sync.dma_start(
                out=vis_sb, in_=vis_d[:, :].rearrange("b (t p) -> p (b t)", p=128)
            )
            q_tm = singles.tile([128, NT, G * 128], bf16)  # rope'd q, token-major
            k_tm = singles.tile([128, NT, 128], bf16)
            vv_sb = singles.tile([128, NT, 129], bf16)  # [v*vis | vis]
            kT_sb = singles.tile([128, 2, S], bf16)  # k transposed per batch

            # ---- stage 1: qkv matmul + rmsnorm + rope (token-major) ----
            for tt in range(NT):
                xt = xin.tile([128, 16, 128], bf16)
                nc.sync.dma_start(
                    out=xt,
                    in_=agx_out[tt // NTS, tt % NTS, :, :].rearrange(
                        "p (c k) -> p c k", c=16
                    ),
                )
                ps = psum_big.tile([128, 768], f32, tag="big")
                for kc in range(16):
                    nc.tensor.matmul(
                        ps[:, 0:512],
                        lhsT=xt[:, kc, :],
                        rhs=wqkv_sb[:, kc, 0:512],
                        start=(kc == 0),
                        stop=(kc == 15),
                    )
                    nc.tensor.matmul(
                        ps[:, 512:768],
                        lhsT=xt[:, kc, :],
                        rhs=wqkv_sb[:, kc, 512:768],
                        start=(kc == 0),
                        stop=(kc == 15),
                    )
                nrm = scr.tile([128, 768], f32, tag="nrm")
                nc.vector.tensor_copy(nrm, ps)
                ss = scr.tile([128, 5], f32, tag="ss")
                stdv = scr.tile([128, 5], f32, tag="stdv")
                rstd = scr.tile([128, 5], f32, tag="rstd")
                sqj = scr.tile([128, 128], f32, tag="sqj")
                t = tt % NTB
                for hh in range(5):
                    sl = nrm[:, hh * 128 : (hh + 1) * 128]
                    nc.scalar.activation(
                        out=sqj,
                        in_=sl,
                        func=mybir.ActivationFunctionType.Square,
                        accum_out=ss[:, hh : hh + 1],
                    )
                    nc.scalar.activation(
                        out=stdv[:, hh : hh + 1],
                        in_=ss[:, hh : hh + 1],
                        func=mybir.ActivationFunctionType.Sqrt,
                        bias=eps_sb[:, :],
                        scale=1.0 / 128.0,
                    )
                    nc.vector.reciprocal(rstd[:, hh : hh + 1], stdv[:, hh : hh + 1])
                    tmp1 = scr.tile([128, 128], f32, tag="tmp1")
                    nc.vector.tensor_scalar_mul(tmp1, sl, rstd[:, hh : hh + 1])
                    t1p = tmp1.rearrange("p (x two) -> p x two", two=2)
                    tmp2 = scr.tile([128, 128], f32, tag="tmp2")
                    t2p = tmp2.rearrange("p (x two) -> p x two", two=2)
                    nc.vector.tensor_scalar_mul(t2p[:, :, 0], t1p[:, :, 1], -1.0)
                    nc.vector.tensor_copy(t2p[:, :, 1], t1p[:, :, 0])
                    cs, sn = (cosq_sb, sinq_sb) if hh < 4 else (cosk_sb, sink_sb)
                    tmp3 = scr.tile([128, 128], f32, tag="tmp3")
                    nc.vector.tensor_mul(tmp3, tmp1, cs[:, t, :])
                    nc.vector.tensor_mul(tmp2, tmp2, sn[:, t, :])
                    dst = (
                        q_tm[:, tt, hh * 128 : (hh + 1) * 128]
                        if hh < 4
                        else k_tm[:, tt, :]
                    )
                    nc.vector.tensor_add(dst, tmp3, tmp2)
                # v with visibility folded in, plus the ones-column
                nc.vector.tensor_scalar_mul(
                    vv_sb[:, tt, 0:128], nrm[:, 640:768], vis_sb[:, tt : tt + 1]
                )

            nc.vector.tensor_copy(
                vv_sb[:, :, 128:129],
                vis_sb[:, :].rearrange("p (t one) -> p t one", one=1),
            )

            # ---- stage 2: transpose K per batch ----
            for b in range(2):
                for kt in range(NTB):
                    pt = psum_t.tile([128, 128], bf16, tag="pt")
                    nc.tensor.matmul(pt, lhsT=k_tm[:, b * NTB + kt, :], rhs=ident, is_transpose=True)
                    nc.vector.tensor_copy(
                        kT_sb[:, b, kt * 128 : (kt + 1) * 128], pt
                    )

            # ---- stage 3+4: attention + output projection per (b, qt) ----
            for b in range(2):
                for qt in range(NTB):
                    tt = b * NTB + qt
                    qT = qtile.tile([128, G, 128], bf16, tag="qT")
                    for hh in range(G):
                        pt = psum_t.tile([128, 128], bf16, tag="pt")
                        nc.tensor.matmul(
                            pt, lhsT=q_tm[:, tt, hh * 128 : (hh + 1) * 128], rhs=ident, is_transpose=True
                        )
                        nc.vector.tensor_copy(qT[:, hh, :], pt)
                    o_tmp = qtile.tile([128, G, 128], bf16, tag="o_tmp")
                    for hh in range(G):
                        p_sb = ptile.tile([128, S], bf16, tag="p_sb")
                        for half in range(2):
                            psc = psum_big.tile([128, 1024], f32, tag="big")
                            for kc in range(2):
                                col = half * 1024 + kc * 512
                                nc.tensor.matmul(
                                    psc[:, kc * 512 : (kc + 1) * 512],
                                    lhsT=qT[:, hh, :],
                                    rhs=kT_sb[:, b, col : col + 512],
                                    start=True,
                                    stop=True,
                                )
                            nc.scalar.activation(
                                out=p_sb[:, half * 1024 : (half + 1) * 1024],
                                in_=psc,
                                func=mybir.ActivationFunctionType.Exp,
                            )
                        pT = ptile.tile([128, NTB, 128], bf16, tag="pT")
                        for kt in range(NTB):
                            pt = psum_t.tile([128, 128], bf16, tag="pt")
                            nc.tensor.matmul(
                                pt, lhsT=p_sb[:, kt * 128 : (kt + 1) * 128], rhs=ident, is_transpose=True
                            )
                            nc.vector.tensor_copy(pT[:, kt, :], pt)
                        po = psum_o.tile([128, 129], f32, tag="po")
                        for kt in range(NTB):
                            nc.tensor.matmul(
                                po,
                                lhsT=pT[:, kt, :],
                                rhs=vv_sb[:, b * NTB + kt, :],
                                start=(kt == 0),
                                stop=(kt == NTB - 1),
                            )
                        rq = scr.tile([128, 1], f32, tag="rq")
                        nc.vector.reciprocal(rq, po[:, 128:129])
                        nc.vector.tensor_scalar_mul(rq, rq, vis_sb[:, tt : tt + 1])
                        nc.vector.tensor_scalar_mul(
                            o_tmp[:, hh, :], po[:, 0:128], rq
                        )
                    # output projection partial: y = o @ woT_slice
                    oT = qtile.tile([128, G, 128], bf16, tag="oT")
                    for ft in range(G):
                        pt = psum_t.tile([128, 128], bf16, tag="pt")
                        nc.tensor.matmul(pt, lhsT=o_tmp[:, ft, :], rhs=ident, is_transpose=True)
                        nc.vector.tensor_copy(oT[:, ft, :], pt)
                    for ncho in range(4):
                        py = psum_o.tile([128, 512], f32, tag="po")
                        for ft in range(4):
                            nc.tensor.matmul(
                                py,
                                lhsT=oT[:, ft, :],
                                rhs=wo_sb[:, ft, ncho * 512 : (ncho + 1) * 512],
                                start=(ft == 0),
                                stop=(ft == 3),
                            )
                        y_sb = scr.tile([128, 512], bf16, tag="y")
                        nc.scalar.copy(y_sb, py)
                        nc.sync.dma_start(
                            out=part_d[
                                tt * 128 : (tt + 1) * 128,
                                ncho * 512 : (ncho + 1) * 512,
                            ],
                            in_=y_sb,
                        )

            # ---- stage 5: ReduceScatter partials, write token-quarter out ----
            nc.gpsimd.collective_compute(
                "ReduceScatter",
                mybir.AluOpType.add,
                replica_groups=GROUPS,
                ins=[part_d.ap().opt()],
                outs=[rs_out.ap().opt()],
            )
            nc.sync.dma_start(out=out_d[:, :], in_=rs_out[:, :])
    nc.finalize()
    return nc


def _prep_inputs(x, wqkv, wo, q_norm_w, k_norm_w, freqs_cos, freqs_sin, vis_mask):
    """Build per-core input maps. Core c: kv head h=c%4, batch pair p=c//4,
    token-quarter r=c%4 of the pair (via AllGather on device)."""
    cos = np.asarray(freqs_cos, np.float32)[:, 0, :]  # [S,128]
    sin = np.asarray(freqs_sin, np.float32)[:, 0, :]
    qw = np.asarray(q_norm_w, np.float32)
    kw = np.asarray(k_norm_w, np.float32)

    def swap_pairs(w):
        v = w.reshape(-1, 2)
        return np.stack([v[:, 1], v[:, 0]], axis=1).reshape(-1)

    cosq = (cos * qw[None, :] * SCALE).astype(BF16)
    sinq = (sin * swap_pairs(qw)[None, :] * SCALE).astype(BF16)
    cosk = (cos * kw[None, :]).astype(BF16)
    sink = (sin * swap_pairs(kw)[None, :]).astype(BF16)

    x = np.asarray(x, np.float32)
    wqkv = np.asarray(wqkv, np.float32)
    wo = np.asarray(wo, np.float32)
    visf = np.asarray(vis_mask).astype(np.float32)

    # per-pair transposed x tiles: [NT, 128 feat, 16*128 tok], computed once
    xT_pairs = []
    vis_pairs = []
    for p in range(2):
        xpair = np.concatenate([x[2 * p], x[2 * p + 1]], axis=0)  # [4096, 2048]
        xT = np.ascontiguousarray(
            xpair.reshape(NT, 128, 16, 128).transpose(0, 3, 2, 1).reshape(NT, 128, 2048)
        ).astype(BF16)
        xT_pairs.append(xT)
        vis_pairs.append(np.ascontiguousarray(visf[2 * p : 2 * p + 2]))

    # per-head weight slices, computed once
    w_slices = []
    for h in range(4):
        wq = wqkv[512 * h : 512 * (h + 1)]  # [512, 2048]
        wk = wqkv[2048 + 128 * h : 2048 + 128 * (h + 1)]
        wv = wqkv[2560 + 128 * h : 2560 + 128 * (h + 1)]
        wslice = np.concatenate([wq, wk, wv], axis=0)  # [768, 2048]
        wqkvT = np.ascontiguousarray(wslice.T).astype(BF16)
        woT = np.ascontiguousarray(wo[:, 512 * h : 512 * (h + 1)].T).astype(BF16)
        w_slices.append((wqkvT, woT))

    in_maps = []
    for c in range(8):
        h = c % 4
        r = c % 4
        p = c // 4
        wqkvT, woT = w_slices[h]
        in_maps.append(
            {
                "xTs": np.ascontiguousarray(xT_pairs[p][NTS * r : NTS * (r + 1)]),
                "wqkvT": wqkvT,
                "woT": woT,
                "cosq": cosq,
                "sinq": sinq,
                "cosk": cosk,
                "sink": sink,
                "vis": vis_pairs[p],
            }
        )
    return in_maps


def run_hw(in_maps, trace=False):
    from concourse.bass_utils import run_bass_kernel_spmd

    if "nc" not in _CACHE:
        _CACHE["nc"] = _build_graph()
    return run_bass_kernel_spmd(
        _CACHE["nc"], in_maps, core_ids=list(range(8)), trace=trace
    )


def kernel(x, wqkv, wo, q_norm_w, k_norm_w, freqs_cos, freqs_sin, vis_mask):
    in_maps = _prep_inputs(
        x, wqkv, wo, q_norm_w, k_norm_w, freqs_cos, freqs_sin, vis_mask
    )
    res = run_hw(in_maps)
    full = np.zeros((B, S, DIM), np.float32)
    Q = TOK // 4  # 1024 tokens per core
    for c in range(8):
        p, r = c // 4, c % 4
        o = np.asarray(res.results[c]["out"], np.float32)  # [1024, 2048]
        b = 2 * p + r // 2
        lo = (r % 2) * Q
        full[b, lo : lo + Q] = o
    return full


# revision 11
# speedup vs baseline: 4.1972x; 1.2272x over previous
import numpy as np
import sys

sys.path.insert(0, "/opt/trn_rl_repo")

import ml_dtypes

BF16 = ml_dtypes.bfloat16

B, S, DIM = 4, 2048, 2048
N_HEADS, N_KV_HEADS, HEAD_DIM = 16, 4, 128
G = N_HEADS // N_KV_HEADS  # 4 q heads per kv head
EPS = 1.1920928955078125e-07
SCALE = 1.0 / np.sqrt(HEAD_DIM)
TOK = 2 * S  # tokens per core group (a batch pair)
NT = TOK // 128  # 32 token tiles per group
NTB = S // 128  # 16 token tiles per batch
NTS = NT // 4  # 8 token tiles per core slice (1024 tokens)
GROUPS = [[0, 1, 2, 3], [4, 5, 6, 7]]
GROUPS2 = [[0, 4], [1, 5], [2, 6], [3, 7]]  # same-head core pairs
GROUPS8 = [[0, 1, 2, 3, 4, 5, 6, 7]]

_CACHE = {}


def _build_graph():
    import concourse.bass as bass
    import concourse.mybir as mybir
    from concourse import bacc
    from concourse.tile import TileContext
    from concourse.masks import make_identity

    f32 = mybir.dt.float32
    bf16 = mybir.dt.bfloat16

    nc = bacc.Bacc()
    # per-core token-quarter of the batch pair, pre-transposed to
    # feature-major tiles: [tile, feature-part, chunk*token]
    xTs_d = nc.declare_dram_parameter("xTs", [NTS, 128, 16 * 128], bf16, isOutput=False)
    wqkvh_d = nc.declare_dram_parameter("wqkvTh", [DIM // 2, 768], bf16, isOutput=False)
    woh_d = nc.declare_dram_parameter("woTh", [256, DIM], bf16, isOutput=False)
    frq_d = nc.declare_dram_parameter("frq", [4, S // 8, 128], bf16, isOutput=False)
    vis_d = nc.declare_dram_parameter("vis", [2, S], f32, isOutput=False)
    out_d = nc.declare_dram_parameter("out", [TOK // 4, DIM], bf16, isOutput=True)

    # DRAM scratch (collectives can't touch I/O tensors)
    agx_in = nc.dram_tensor("agx_in", [NTS, 128, 16 * 128], bf16)
    agx_out = nc.dram_tensor("agx_out", [4, NTS, 128, 16 * 128], bf16)
    agw_in = nc.dram_tensor("agw_in", [DIM // 2, 768], bf16)
    agw_out = nc.dram_tensor("agw_out", [2, DIM // 2, 768], bf16)
    agwo_in = nc.dram_tensor("agwo_in", [256, DIM], bf16)
    agwo_out = nc.dram_tensor("agwo_out", [2, 256, DIM], bf16)
    agf_in = nc.dram_tensor("agf_in", [4, S // 8, 128], bf16)
    agf_out = nc.dram_tensor("agf_out", [8, 4, S // 8, 128], bf16, addr_space="Shared")
    part_d = nc.dram_tensor("part", [TOK, DIM], bf16)
    rs_out = nc.dram_tensor("rs_out", [TOK // 4, DIM], bf16)

    with TileContext(nc) as tc:
        with (
            tc.tile_pool(name="singles", bufs=1) as singles,
            tc.tile_pool(name="xin", bufs=2) as xin,
            tc.tile_pool(name="scr", bufs=2) as scr,
            tc.tile_pool(name="ptile", bufs=2) as ptile,
            tc.tile_pool(name="qtile", bufs=2) as qtile,
            tc.tile_pool(name="psum_big", bufs=2, space="PSUM") as psum_big,
            tc.tile_pool(name="psum_t", bufs=2, space="PSUM") as psum_t,
            tc.tile_pool(name="psum_o", bufs=2, space="PSUM") as psum_o,
        ):
            # ---- stage 0: AllGather sharded inputs ----
            # x token-quarters across the 4-core pair group
            nc.sync.dma_start(out=agx_in[:, :, :], in_=xTs_d[:, :, :])
            nc.gpsimd.collective_compute(
                "AllGather",
                mybir.AluOpType.bypass,
                replica_groups=GROUPS,
                ins=[agx_in.ap().opt()],
                outs=[agx_out.ap().opt()],
            )
            # weight halves across same-head core pairs
            nc.sync.dma_start(out=agw_in[:, :], in_=wqkvh_d[:, :])
            nc.gpsimd.collective_compute(
                "AllGather",
                mybir.AluOpType.bypass,
                replica_groups=GROUPS2,
                ins=[agw_in.ap().opt()],
                outs=[agw_out.ap().opt()],
            )
            nc.sync.dma_start(out=agwo_in[:, :], in_=woh_d[:, :])
            nc.gpsimd.collective_compute(
                "AllGather",
                mybir.AluOpType.bypass,
                replica_groups=GROUPS2,
                ins=[agwo_in.ap().opt()],
                outs=[agwo_out.ap().opt()],
            )
            # freq-table eighths across all 8 cores
            nc.sync.dma_start(out=agf_in[:, :, :], in_=frq_d[:, :, :])
            nc.gpsimd.collective_compute(
                "AllGather",
                mybir.AluOpType.bypass,
                replica_groups=GROUPS8,
                ins=[agf_in.ap().opt()],
                outs=[agf_out.ap().opt()],
            )

            # ---- resident tiles ----
            ident = singles.tile([128, 128], bf16)
            make_identity(nc, ident)
            eps_sb = singles.tile([128, 1], f32)
            nc.vector.memset(eps_sb, EPS)
            wqkv_sb = singles.tile([128, 16, 768], bf16)
            nc.sync.dma_start(
                out=wqkv_sb,
                in_=agw_out[:, :, :].rearrange("r (c p) f -> p (r c) f", p=128),
            )
            wo_sb = singles.tile([128, 4, DIM], bf16)
            nc.sync.dma_start(
                out=wo_sb,
                in_=agwo_out[:, :, :].rearrange("r (c p) f -> p (r c) f", p=128),
            )
            cosq_sb = singles.tile([128, NTB, 128], bf16)
            sinq_sb = singles.tile([128, NTB, 128], bf16)
            cosk_sb = singles.tile([128, NTB, 128], bf16)
            sink_sb = singles.tile([128, NTB, 128], bf16)
            for j, sb in enumerate((cosq_sb, sinq_sb, cosk_sb, sink_sb)):
                sbv = sb.rearrange("p (r t) d -> p r t d", r=8)
                for t in range(2):
                    nc.sync.dma_start(
                        out=sbv[:, :, t, :],
                        in_=agf_out[:, j, 128 * t : 128 * (t + 1), :].rearrange(
                            "r p d -> p r d"
                        ),
                    )
            vis_sb = singles.tile([128, NT], f32)
            nc.sync.dma_start(
                out=vis_sb, in_=vis_d[:, :].rearrange("b (t p) -> p (b t)", p=128)
            )
            q_tm = singles.tile([128, NT, G * 128], bf16)  # rope'd q, token-major
            k_tm = singles.tile([128, NT, 128], bf16)
            vv_sb = singles.tile([128, NT, 129], bf16)  # [v*vis | vis]
            kT_sb = singles.tile([128, 2, S], bf16)  # k transposed per batch

            # ---- stage 1: qkv matmul + rmsnorm + rope (token-major) ----
            for tt in range(NT):
                xt = xin.tile([128, 16, 128], bf16)
                nc.sync.dma_start(
                    out=xt,
                    in_=agx_out[tt // NTS, tt % NTS, :, :].rearrange(
                        "p (c k) -> p c k", c=16
                    ),
                )
                ps = psum_big.tile([128, 768], f32, tag="big")
                for kc in range(16):
                    nc.tensor.matmul(
                        ps[:, 0:512],
                        lhsT=xt[:, kc, :],
                        rhs=wqkv_sb[:, kc, 0:512],
                        start=(kc == 0),
                        stop=(kc == 15),
                    )
                    nc.tensor.matmul(
                        ps[:, 512:768],
                        lhsT=xt[:, kc, :],
                        rhs=wqkv_sb[:, kc, 512:768],
                        start=(kc == 0),
                        stop=(kc == 15),
                    )
                nrm = scr.tile([128, 768], f32, tag="nrm")
                nc.vector.tensor_copy(nrm, ps)
                ss = scr.tile([128, 5], f32, tag="ss")
                stdv = scr.tile([128, 5], f32, tag="stdv")
                rstd = scr.tile([128, 5], f32, tag="rstd")
                sqj = scr.tile([128, 128], f32, tag="sqj")
                t = tt % NTB
                for hh in range(5):
                    sl = nrm[:, hh * 128 : (hh + 1) * 128]
                    nc.scalar.activation(
                        out=sqj,
                        in_=sl,
                        func=mybir.ActivationFunctionType.Square,
                        accum_out=ss[:, hh : hh + 1],
                    )
                    nc.scalar.activation(
                        out=stdv[:, hh : hh + 1],
                        in_=ss[:, hh : hh + 1],
                        func=mybir.ActivationFunctionType.Sqrt,
                        bias=eps_sb[:, :],
                        scale=1.0 / 128.0,
                    )
                    nc.vector.reciprocal(rstd[:, hh : hh + 1], stdv[:, hh : hh + 1])
                    tmp1 = scr.tile([128, 128], f32, tag="tmp1")
                    nc.vector.tensor_scalar_mul(tmp1, sl, rstd[:, hh : hh + 1])
                    t1p = tmp1.rearrange("p (x two) -> p x two", two=2)
                    tmp2 = scr.tile([128, 128], f32, tag="tmp2")
                    t2p = tmp2.rearrange("p (x two) -> p x two", two=2)
                    nc.vector.tensor_scalar_mul(t2p[:, :, 0], t1p[:, :, 1], -1.0)
                    nc.vector.tensor_copy(t2p[:, :, 1], t1p[:, :, 0])
                    cs, sn = (cosq_sb, sinq_sb) if hh < 4 else (cosk_sb, sink_sb)
                    tmp3 = scr.tile([128, 128], f32, tag="tmp3")
                    nc.vector.tensor_mul(tmp3, tmp1, cs[:, t, :])
                    nc.vector.tensor_mul(tmp2, tmp2, sn[:, t, :])
                    dst = (
                        q_tm[:, tt, hh * 128 : (hh + 1) * 128]
                        if hh < 4
                        else k_tm[:, tt, :]
                    )
                    nc.vector.tensor_add(dst, tmp3, tmp2)
                # v with visibility folded in, plus the ones-column
                nc.vector.tensor_scalar_mul(
                    vv_sb[:, tt, 0:128], nrm[:, 640:768], vis_sb[:, tt : tt + 1]
                )

            nc.vector.tensor_copy(
                vv_sb[:, :, 128:129],
                vis_sb[:, :].rearrange("p (t one) -> p t one", one=1),
            )

            # ---- stage 2: transpose K per batch ----
            for b in range(2):
                for kt in range(NTB):
                    pt = psum_t.tile([128, 128], bf16, tag="pt")
                    nc.tensor.matmul(pt, lhsT=k_tm[:, b * NTB + kt, :], rhs=ident, is_transpose=True)
                    nc.vector.tensor_copy(
                        kT_sb[:, b, kt * 128 : (kt + 1) * 128], pt
                    )

            # ---- stage 3+4: attention + output projection per (b, qt) ----
            for b in range(2):
                for qt in range(NTB):
                    tt = b * NTB + qt
                    qT = qtile.tile([128, G, 128], bf16, tag="qT")
                    for hh in range(G):
                        pt = psum_t.tile([128, 128], bf16, tag="pt")
                        nc.tensor.matmul(
                            pt, lhsT=q_tm[:, tt, hh * 128 : (hh + 1) * 128], rhs=ident, is_transpose=True
                        )
                        nc.vector.tensor_copy(qT[:, hh, :], pt)
                    o_tmp = qtile.tile([128, G, 128], bf16, tag="o_tmp")
                    for hh in range(G):
                        p_sb = ptile.tile([128, S], bf16, tag="p_sb")
                        for half in range(2):
                            psc = psum_big.tile([128, 1024], f32, tag="big")
                            for kc in range(2):
                                col = half * 1024 + kc * 512
                                nc.tensor.matmul(
                                    psc[:, kc * 512 : (kc + 1) * 512],
                                    lhsT=qT[:, hh, :],
                                    rhs=kT_sb[:, b, col : col + 512],
                                    start=True,
                                    stop=True,
                                )
                            nc.scalar.activation(
                                out=p_sb[:, half * 1024 : (half + 1) * 1024],
                                in_=psc,
                                func=mybir.ActivationFunctionType.Exp,
                            )
                        pT = ptile.tile([128, NTB, 128], bf16, tag="pT")
                        for kt in range(NTB):
                            pt = psum_t.tile([128, 128], bf16, tag="pt")
                            nc.tensor.matmul(
                                pt, lhsT=p_sb[:, kt * 128 : (kt + 1) * 128], rhs=ident, is_transpose=True
                            )
                            nc.vector.tensor_copy(pT[:, kt, :], pt)
                        po = psum_o.tile([128, 129], f32, tag="po")
                        for kt in range(NTB):
                            nc.tensor.matmul(
                                po,
                                lhsT=pT[:, kt, :],
                                rhs=vv_sb[:, b * NTB + kt, :],
                                start=(kt == 0),
                                stop=(kt == NTB - 1),
                            )
                        rq = scr.tile([128, 1], f32, tag="rq")
                        nc.vector.reciprocal(rq, po[:, 128:129])
                        nc.vector.tensor_scalar_mul(rq, rq, vis_sb[:, tt : tt + 1])
                        nc.vector.tensor_scalar_mul(
                            o_tmp[:, hh, :], po[:, 0:128], rq
                        )
                    # output projection partial: y = o @ woT_slice
                    oT = qtile.tile([128, G, 128], bf16, tag="oT")
                    for ft in range(G):
                        pt = psum_t.tile([128, 128], bf16, tag="pt")
                        nc.tensor.matmul(pt, lhsT=o_tmp[:, ft, :], rhs=ident, is_transpose=True)
                        nc.vector.tensor_copy(oT[:, ft, :], pt)
                    for ncho in range(4):
                        py = psum_o.tile([128, 512], f32, tag="po")
                        for ft in range(4):
                            nc.tensor.matmul(
                                py,
                                lhsT=oT[:, ft, :],
                                rhs=wo_sb[:, ft, ncho * 512 : (ncho + 1) * 512],
                                start=(ft == 0),
                                stop=(ft == 3),
                            )
                        y_sb = scr.tile([128, 512], bf16, tag="y")
                        nc.scalar.copy(y_sb, py)
                        nc.sync.dma_start(
                            out=part_d[
                                tt * 128 : (tt + 1) * 128,
                                ncho * 512 : (ncho + 1) * 512,
                            ],
                            in_=y_sb,
                        )

            # ---- stage 5: ReduceScatter partials, write token-quarter out ----
            nc.gpsimd.collective_compute(
                "ReduceScatter",
                mybir.AluOpType.add,
                replica_groups=GROUPS,
                ins=[part_d.ap().opt()],
                outs=[rs_out.ap().opt()],
            )
            nc.sync.dma_start(out=out_d[:, :], in_=rs_out[:, :])
    nc.finalize()
    return nc


def _prep_inputs(x, wqkv, wo, q_norm_w, k_norm_w, freqs_cos, freqs_sin, vis_mask):
    """Build per-core input maps. Core c: kv head h=c%4, batch pair p=c//4,
    token-quarter r=c%4 of the pair (via AllGather on device)."""
    cos = np.asarray(freqs_cos, np.float32)[:, 0, :]  # [S,128]
    sin = np.asarray(freqs_sin, np.float32)[:, 0, :]
    qw = np.asarray(q_norm_w, np.float32)
    kw = np.asarray(k_norm_w, np.float32)

    def swap_pairs(w):
        v = w.reshape(-1, 2)
        return np.stack([v[:, 1], v[:, 0]], axis=1).reshape(-1)

    cosq = (cos * qw[None, :] * SCALE).astype(BF16)
    sinq = (sin * swap_pairs(qw)[None, :] * SCALE).astype(BF16)
    cosk = (cos * kw[None, :]).astype(BF16)
    sink = (sin * swap_pairs(kw)[None, :]).astype(BF16)

    x = np.asarray(x, np.float32)
    wqkv = np.asarray(wqkv, np.float32)
    wo = np.asarray(wo, np.float32)
    visf = np.asarray(vis_mask).astype(np.float32)

    # per-pair transposed x tiles: [NT, 128 feat, 16*128 tok], computed once
    xT_pairs = []
    vis_pairs = []
    for p in range(2):
        xpair = np.concatenate([x[2 * p], x[2 * p + 1]], axis=0)  # [4096, 2048]
        xT = np.ascontiguousarray(
            xpair.reshape(NT, 128, 16, 128).transpose(0, 3, 2, 1).reshape(NT, 128, 2048)
        ).astype(BF16)
        xT_pairs.append(xT)
        vis_pairs.append(np.ascontiguousarray(visf[2 * p : 2 * p + 2]))

    # per-head weight slices, computed once
    w_slices = []
    for h in range(4):
        wq = wqkv[512 * h : 512 * (h + 1)]  # [512, 2048]
        wk = wqkv[2048 + 128 * h : 2048 + 128 * (h + 1)]
        wv = wqkv[2560 + 128 * h : 2560 + 128 * (h + 1)]
        wslice = np.concatenate([wq, wk, wv], axis=0)  # [768, 2048]
        wqkvT = np.ascontiguousarray(wslice.T).astype(BF16)  # [2048, 768]
        woT = np.ascontiguousarray(wo[:, 512 * h : 512 * (h + 1)].T).astype(BF16)
        w_slices.append((wqkvT, woT))

    frq = np.stack([cosq, sinq, cosk, sink])  # [4, 2048, 128]

    in_maps = []
    for c in range(8):
        h = c % 4
        r = c % 4
        p = c // 4
        half = c // 4  # rank within the same-head pair (c, c+4)
        wqkvT, woT = w_slices[h]
        E = S // 8
        in_maps.append(
            {
                "xTs": np.ascontiguousarray(xT_pairs[p][NTS * r : NTS * (r + 1)]),
                "wqkvTh": np.ascontiguousarray(
                    wqkvT[1024 * half : 1024 * (half + 1)]
                ),
                "woTh": np.ascontiguousarray(woT[256 * half : 256 * (half + 1)]),
                "frq": np.ascontiguousarray(frq[:, E * c : E * (c + 1), :]),
                "vis": vis_pairs[p],
            }
        )
    return in_maps


def run_hw(in_maps, trace=False):
    from concourse.bass_utils import run_bass_kernel_spmd

    if "nc" not in _CACHE:
        _CACHE["nc"] = _build_graph()
    return run_bass_kernel_spmd(
        _CACHE["nc"], in_maps, core_ids=list(range(8)), trace=trace
    )


def kernel(x, wqkv, wo, q_norm_w, k_norm_w, freqs_cos, freqs_sin, vis_mask):
    in_maps = _prep_inputs(
        x, wqkv, wo, q_norm_w, k_norm_w, freqs_cos, freqs_sin, vis_mask
    )
    res = run_hw(in_maps)
    full = np.zeros((B, S, DIM), np.float32)
    Q = TOK // 4  # 1024 tokens per core
    for c in range(8):
        p, r = c // 4, c % 4
        o = np.asarray(res.results[c]["out"], np.float32)  # [1024, 2048]
        b = 2 * p + r // 2
        lo = (r % 2) * Q
        full[b, lo : lo + Q] = o
    return full
